# revision 1
# baseline (speedup 1.0000x reference)
"""Trainium2 Bass kernel for causal self-attention (B=2, T=2048, C=1024, H=16).

Sharding: tensor-parallel over heads x data-parallel over batch.
Each of the 8 cores handles one (batch b, head-group g) pair: b = core // 4,
g = core % 4, where a head group is 4 consecutive heads (heads 4g..4g+3).

Per-core pipeline:
  1. QKV projection from host-pre-transposed xT [C, T]:
       qT/kT per head-pair [128, T]  (partitions = 2 heads x 64 dims)
       v per head [128, 16*65]       (T-blocks of 128 on partitions; 65th
                                      column per block is 1.0 -> row sums)
  2. Attention per head in transposed layout: S^T[k, q] = kT.T @ qT blocks.
     The two heads of a pair run as interleaved chains whose S matmuls sit
     in different PE row groups (base partitions 0/64) and execute
     concurrently in the array. exp on ACT straight out of PSUM; causal
     masking by 0/1 mask multiplies on DVE; yT[d, q] accumulated as
     v_aug.T @ P^T (row 64 = softmax denominator l).
  3. Normalize: l -> SBUF (ACT), partition-broadcast on GPSIMD,
     reciprocal_approx_fast + multiply on DVE.
  4. Output projection: out[t, cout] partial = yT.T @ Wp rows; partials from
     the 4 head-groups of a batch are summed on the host (the TP all-reduce).

All SBUF pools stay open for the whole kernel (everything fits), so phases
overlap freely; only PSUM pools are scoped.

Matmul dtype configurable: KBASS_CFG in {f32r, attn_bf16, bf16}.
"""

import os
import numpy as np
from contextlib import ExitStack

import concourse.bass as bass
import concourse.tile as tile
from concourse import bacc, library_config, mybir
from concourse.bass import ts
from concourse.bass_utils import run_bass_kernel_spmd

F32 = mybir.dt.float32
F32R = mybir.dt.float32r
BF16 = mybir.dt.bfloat16
AF = mybir.ActivationFunctionType
PSUM = bass.MemorySpace.PSUM

B, T, C, H = 2, 2048, 1024, 16
HD = C // H              # 64
HPC = 4                  # heads per core
PAIRS = 2                # head pairs per core
CI = C // 128            # 8 contraction chunks
TB = T // 128            # 16 t-blocks
NQC = T // 512           # 4 q-chunks
N_CORES = 8

CFG = os.environ.get("KBASS_CFG", "bf16")
if CFG == "bf16":
    IO_DT = BF16          # xT / weights dram+sbuf
    QKV_DT = BF16         # qT/kT/v tiles
    P_DT = BF16           # exp output tiles
    Y_DT = BF16           # normalized yT tiles
elif CFG == "attn_bf16":
    IO_DT = F32R
    QKV_DT = BF16
    P_DT = BF16
    Y_DT = F32R
else:
    IO_DT = F32R
    QKV_DT = F32R
    P_DT = F32R
    Y_DT = F32R

# 1/l broadcast: gpsimd partition_broadcast (default) or PE outer product
GP_BCAST = os.environ.get("KBASS_GP_BCAST", "1") == "1"


def _emit(tc, nc, xT_d, wq_d, wk_d, wv_d, wp_d, out_d):
    ctx = ExitStack()
    with ctx:
        pers = ctx.enter_context(tc.tile_pool(name="pers", bufs=1))
        if GP_BCAST:
            nc.gpsimd.load_library(library_config.attn)

        qT = [pers.tile([128, T], QKV_DT, name=f"qT{p}") for p in range(PAIRS)]
        kT = [pers.tile([128, T], QKV_DT, name=f"kT{p}") for p in range(PAIRS)]
        v_sb = [pers.tile([128, TB * 65], QKV_DT, name=f"v{h}") for h in range(HPC)]
        yT = [pers.tile([128, T], Y_DT, name=f"yT{p}") for p in range(PAIRS)]
        wp_sb = pers.tile([128, 2048], IO_DT, name="wp")
        ones_sb = pers.tile([1, 64], F32R, name="ones")
        # 0/1 causal mask for the diagonal 128-block: (y - x >= 0)
        mask_d = pers.tile([128, 128], P_DT, name="mask_d")

        nc.sync.dma_start(wp_sb[:], wp_d[:])
        ones_f = pers.tile([128, 1], F32, name="ones_f")
        nc.gpsimd.memset(ones_f[:], 1.0)
        nc.vector.tensor_copy(ones_sb[:], ones_f[0:1, 0:1].broadcast_to([1, 64]))
        for h in range(HPC):
            # 1.0 into column 64 of every 65-wide t-block (softmax denominator)
            nc.vector.tensor_copy(
                v_sb[h][:].rearrange("p (t c) -> p t c", c=65)[:, :, 64:65],
                ones_f[:].unsqueeze(1).broadcast_to([128, TB, 1]),
            )
        mask_f = pers.tile([128, 128], F32, name="mask_f")
        nc.gpsimd.memset(mask_f[:], 1.0)
        nc.gpsimd.affine_select(
            out=mask_f[:], in_=mask_f[:],
            compare_op=mybir.AluOpType.is_ge, fill=0.0,
            base=0, channel_multiplier=-1, pattern=[[1, 128]],
        )
        nc.vector.tensor_copy(mask_d[:], mask_f[:])

        # weights for pair 0 first — they gate the first matmuls
        wq_sb = [pers.tile([128, 1024], IO_DT, name=f"wq{p}") for p in range(PAIRS)]
        wk_sb = [pers.tile([128, 1024], IO_DT, name=f"wk{p}") for p in range(PAIRS)]
        wv_sb = pers.tile([128, 2048], IO_DT, name="wv")
        nc.sync.dma_start(wk_sb[0][:], wk_d[0])
        nc.sync.dma_start(wq_sb[0][:], wq_d[0])
        xT_tiles = [pers.tile([128, T], IO_DT, name=f"xt{ci}") for ci in range(CI)]
        for ci in range(CI):
            nc.sync.dma_start(xT_tiles[ci][:], xT_d[ts(ci, 128), :])
        xT_sb = [t[:] for t in xT_tiles]
        nc.sync.dma_start(wv_sb[:], wv_d[:])
        nc.sync.dma_start(wq_sb[1][:], wq_d[1])
        nc.sync.dma_start(wk_sb[1][:], wk_d[1])

        # ------------- Fully interleaved pipeline -------------
        # Emission order produces QKV chunks just-in-time for attention:
        #   for qc: [k0(qc), q0(qc), v(4qc..4qc+3)] then attention(p0, qc)
        #           (with one qk-pair1 chain folded into every kb step)
        #   then:   attention(p1, qc) + projection t-blocks 4qc..4qc+3
        # PSUM bank plan: psA(qk+v chains) 2, psS(sp0/sp1) 4, psY(ypt0/1) 2;
        # psA's banks are handed to psO (projection) for the second half.

        def emit_qk_chain(w_sb, dst, qc, pool):
            ps = pool.tile([128, 512], F32, tag="psqk", name="psqk")
            for ci in range(CI):
                nc.tensor.matmul(
                    ps[:], w_sb[:, ts(ci, 128)], xT_sb[ci][:, ts(qc, 512)],
                    start=(ci == 0), stop=(ci == CI - 1),
                )
            nc.vector.tensor_copy(dst[:, ts(qc, 512)], ps[:])

        def emit_v(tb, pool):
            # shares the qk chains' double-buffered slot (sized [128, 512])
            psv = pool.tile([128, 256], F32, tag="psqk", name="psv")
            for ci in range(CI):
                nc.tensor.matmul(
                    psv[:], xT_sb[ci][:, ts(tb, 128)], wv_sb[:, ts(ci, 256)],
                    start=(ci == 0), stop=(ci == CI - 1),
                )
            for h in range(HPC):
                nc.vector.tensor_copy(
                    v_sb[h][:, tb * 65: tb * 65 + 64], psv[:, ts(h, 64)]
                )

        def attn_qc(p, qc, psS, psY, pP, pN, filler):
            ypt2 = [psY.tile([128, 512], F32, tag=f"ypt{hh}", bufs=1,
                             name=f"ypt_p{p}q{qc}h{hh}") for hh in (0, 1)]
            nkb = 4 * qc + 4     # causal: k-blocks 0 .. 4*qc+3
            for kb0 in range(0, nkb, 2):
                if filler is not None:
                    filler()
                sps = [psS.tile([128, 1024], F32, tag=f"sp{hh}", bufs=1,
                                name=f"sp{hh}") for hh in (0, 1)]
                # S matmuls; diagonal k-blocks skip cols q < kb*128
                for j in (0, 1):
                    kb = kb0 + j
                    col = max(0, (kb - 4 * qc) * 128)
                    for hh in (0, 1):
                        off = hh * 64
                        nc.tensor.matmul(
                            sps[hh][:, j * 512 + col: (j + 1) * 512],
                            kT[p][off:off + 64, ts(kb, 128)],
                            qT[p][off:off + 64, qc * 512 + col: (qc + 1) * 512],
                            start=True, stop=True,
                        )
                pts = []
                for hh in (0, 1):
                    pt = pP.tile([128, 1024], P_DT, tag=f"pt{hh}", name=f"pt{hh}")
                    nc.scalar.activation(pt[:], sps[hh][:], AF.Exp)
                    for j in (0, 1):
                        kb = kb0 + j
                        if kb >= 4 * qc:   # mask the diagonal 128-block
                            col = j * 512 + (kb - 4 * qc) * 128
                            nc.vector.tensor_mul(
                                pt[:, col:col + 128], pt[:, col:col + 128],
                                mask_d[:],
                            )
                    pts.append(pt)
                for j in (0, 1):
                    kb = kb0 + j
                    col = max(0, (kb - 4 * qc) * 128)
                    for hh in (0, 1):
                        nc.tensor.matmul(
                            ypt2[hh][0:65, col:512],
                            v_sb[2 * p + hh][:, kb * 65:(kb + 1) * 65],
                            pts[hh][:, j * 512 + col: (j + 1) * 512],
                            start=(kb == 0), stop=(kb == nkb - 1),
                        )
            # normalize: yT = num * (1/l), multiplying straight out of PSUM
            for hh in (0, 1):
                off = hh * 64
                l_sb = pN.tile([1, 512], F32, tag="lr")
                nc.scalar.copy(l_sb[:], ypt2[hh][64:65, :])
                if GP_BCAST:
                    lb = pN.tile([64, 512], F32, tag="lb")
                    nc.gpsimd.partition_broadcast(lb[:], l_sb[:])
                else:
                    lbp = psY.tile([64, 512], F32, tag="bl", bufs=1)
                    nc.tensor.matmul(lbp[:], ones_sb[:], l_sb[:].bitcast(F32R),
                                     start=True, stop=True)
                    lb = pN.tile([64, 512], F32, tag="lb")
                    nc.vector.tensor_copy(lb[:], lbp[:])
                rl = pN.tile([64, 512], F32, tag="rl")
                nc.vector.reciprocal_approx_fast(rl[:], lb[:])
                nc.vector.tensor_mul(
                    yT[p][off:off + 64, ts(qc, 512)], ypt2[hh][0:64, :], rl[:],
                )

        with (
            tc.tile_pool(name="psS", bufs=1, space=PSUM) as psS,
            tc.tile_pool(name="psY", bufs=1, space=PSUM) as psY,
            tc.tile_pool(name="pP", bufs=4) as pP,
            tc.tile_pool(name="pN", bufs=3) as pN,
        ):
            # qk pair-1 chains, folded one-per-kb-step into attention p0
            def qk1_chains():
                for w_sb, dst in ((wk_sb[1], kT[1]), (wq_sb[1], qT[1])):
                    for qc in range(NQC):
                        yield w_sb, dst, qc
            qk1 = iter(qk1_chains())

            with tc.tile_pool(name="psA", bufs=2, space=PSUM) as psA:
                def filler():
                    nxt = next(qk1, None)
                    if nxt is not None:
                        w_sb, dst, qc = nxt
                        emit_qk_chain(w_sb, dst, qc, psA)

                for qc in range(NQC):
                    emit_qk_chain(wk_sb[0], kT[0], qc, psA)
                    emit_qk_chain(wq_sb[0], qT[0], qc, psA)
                    for tb in range(4 * qc, 4 * qc + 4):
                        emit_v(tb, psA)
                    attn_qc(0, qc, psS, psY, pP, pN, filler)
                # drain any remaining pair-1 chains
                while True:
                    nxt = next(qk1, None)
                    if nxt is None:
                        break
                    w_sb, dst, qc = nxt
                    emit_qk_chain(w_sb, dst, qc, psA)

            # attention pair 1, with projection t-blocks folded in
            with (
                tc.tile_pool(name="psO", bufs=2, space=PSUM) as psO,
                tc.tile_pool(name="pO", bufs=3) as pO,
            ):
                def emit_proj_chunk(tb, cc):
                    po = psO.tile([128, 512], F32, tag="po", name="po")
                    for p in range(PAIRS):
                        nc.tensor.matmul(
                            po[:], yT[p][:, ts(tb, 128)],
                            wp_sb[:, p * 1024 + cc * 512:
                                  p * 1024 + cc * 512 + 512],
                            start=(p == 0), stop=(p == PAIRS - 1),
                        )
                    ot = pO.tile([128, 512], F32, tag="ot", name="ot")
                    nc.vector.tensor_copy(ot[:], po[:])
                    nc.sync.dma_start(out_d[ts(tb, 128), ts(cc, 512)], ot[:])

                # projection chunks trail the attention by one q-chunk so
                # they never wait on a just-finished normalize
                proj_queue = []

                def proj_filler():
                    for _ in range(2):
                        if proj_queue:
                            emit_proj_chunk(*proj_queue.pop(0))

                for qc in range(NQC):
                    attn_qc(1, qc, psS, psY, pP, pN,
                            proj_filler if qc > 0 else None)
                    proj_queue += [(tb, cc) for tb in range(4 * qc, 4 * qc + 4)
                                   for cc in range(2)]
                while proj_queue:
                    emit_proj_chunk(*proj_queue.pop(0))


_NC_CACHE = None


def _build():
    global _NC_CACHE
    if _NC_CACHE is not None:
        return _NC_CACHE
    nc = bacc.Bacc("TRN2", target_bir_lowering=False, debug=False,
                   num_devices=N_CORES)
    xT_d = nc.dram_tensor("xT", [C, T], IO_DT, kind="ExternalInput")
    wq_d = nc.dram_tensor("wq", [PAIRS, 128, 1024], IO_DT, kind="ExternalInput")
    wk_d = nc.dram_tensor("wk", [PAIRS, 128, 1024], IO_DT, kind="ExternalInput")
    wv_d = nc.dram_tensor("wv", [128, 2048], IO_DT, kind="ExternalInput")
    wp_d = nc.dram_tensor("wp", [128, 2048], IO_DT, kind="ExternalInput")
    out_d = nc.dram_tensor("out", [T, C], F32, kind="ExternalOutput")

    with tile.TileContext(nc) as tc:
        _emit(tc, nc, xT_d, wq_d, wk_d, wv_d, wp_d, out_d)
    nc.compile()
    _NC_CACHE = nc
    return nc


def _pack_pair(m):
    # [1024, 128] -> lhsT chunks layout [128, 8*128]
    return np.ascontiguousarray(
        m.reshape(CI, 128, 128).transpose(1, 0, 2).reshape(128, 1024))


def _io_np(a):
    if IO_DT == BF16:
        import ml_dtypes
        return np.ascontiguousarray(a.astype(ml_dtypes.bfloat16))
    return np.ascontiguousarray(a.astype(np.float32))


def _in_maps(x, w_attn, w_proj):
    x = np.asarray(x, dtype=np.float32)
    w_attn = np.asarray(w_attn, dtype=np.float32)
    w_proj = np.asarray(w_proj, dtype=np.float32)
    xT = [_io_np(x[b].T) for b in range(B)]
    maps = []
    for core in range(N_CORES):
        b, g = core // HPC, core % HPC
        cols = slice(g * 256, (g + 1) * 256)
        wk_full = w_attn[:, 0 * C:1 * C][:, cols]
        wq_full = w_attn[:, 1 * C:2 * C][:, cols] * np.float32(1.0 / np.sqrt(HD))
        wv_full = w_attn[:, 2 * C:3 * C][:, cols]
        wq_in = np.stack([_pack_pair(wq_full[:, p * 128:(p + 1) * 128])
                          for p in range(PAIRS)])
        wk_in = np.stack([_pack_pair(wk_full[:, p * 128:(p + 1) * 128])
                          for p in range(PAIRS)])
        wv_in = wv_full.reshape(CI, 128, 256).transpose(1, 0, 2).reshape(128, 2048)
        wp_in = (w_proj[g * 256:(g + 1) * 256, :]
                 .reshape(PAIRS, 128, 1024).transpose(1, 0, 2).reshape(128, 2048))
        maps.append({"xT": xT[b], "wq": _io_np(wq_in), "wk": _io_np(wk_in),
                     "wv": _io_np(wv_in), "wp": _io_np(wp_in)})
    return maps


def _assemble(results, b_proj):
    b_proj = np.asarray(b_proj, dtype=np.float32)
    out = np.zeros((B, T, C), dtype=np.float32)
    for core in range(N_CORES):
        out[core // HPC] += results[core]["out"]
    out += b_proj[None, None, :]
    return out


def kernel(x, w_attn, w_proj, b_proj):
    nc = _build()
    maps = _in_maps(x, w_attn, w_proj)
    res = run_bass_kernel_spmd(nc, maps, list(range(N_CORES)))
    return _assemble(res.results, b_proj)


def kernel_traced(x, w_attn, w_proj, b_proj):
    """Like kernel() but with NTFF tracing; returns (out, BassKernelResults)."""
    nc = _build()
    maps = _in_maps(x, w_attn, w_proj)
    res = run_bass_kernel_spmd(nc, maps, list(range(N_CORES)), trace=True)
    return _assemble(res.results, b_proj), res



# revision 20
# speedup vs baseline: 1.0239x; 1.0239x over previous
"""Trainium2 Bass kernel for causal self-attention (B=2, T=2048, C=1024, H=16).

Sharding: tensor-parallel over heads x data-parallel over batch.
Each of the 8 cores handles one (batch b, head-group g) pair: b = core // 4,
g = core % 4, where a head group is 4 consecutive heads (heads 4g..4g+3).

Per-core pipeline (v2 — software-pipelined, PE-saturating):
  Ramp: ci-major qkv chains (k0/q0/k1/q1 for qc0 + v tb0..3) so the PE
        computes while the 4MB xT streams in.
  Attention per (pair, qc), one k-block per step, AV lagging S by one step:
        PE order: S(kb+1) | filler | AV(kb); exp(kb+1) on ACT overlaps.
        Both heads' S^T live in one [128,1024] PSUM tile -> single exp.
  l-broadcast for free: v_aug columns 64..127 are 1.0, so AV's PSUM rows
        64..127 hold the softmax denominator replicated across partitions;
        normalize = DVE reciprocal + multiply straight out of PSUM.
  Fillers: remaining qk/v chains (phase 1) and projection chunks (phase 2)
        are interleaved between S and AV to hide exp latency and keep the
        PE p-state at max clock.
  Output projection partials summed on the host (the TP all-reduce), plus
        b_proj.
"""

import os
import numpy as np
from contextlib import ExitStack

DEBUG_V = os.environ.get("KBASS_DEBUG_V", "0") == "1"

import concourse.bass as bass
import concourse.tile as tile
from concourse import bacc, library_config, mybir
from concourse.bass import ts
from concourse.bass_utils import run_bass_kernel_spmd

F32 = mybir.dt.float32
BF16 = mybir.dt.bfloat16
AF = mybir.ActivationFunctionType
PSUM = bass.MemorySpace.PSUM

B, T, C, H = 2, 2048, 1024, 16
HD = C // H              # 64
HPC = 4                  # heads per core
PAIRS = 2                # head pairs per core
CI = C // 128            # 8 contraction chunks
TB = T // 128            # 16 t-blocks
NQC = T // 512           # 4 q-chunks
N_CORES = 8

IO_DT = BF16
QKV_DT = BF16
P_DT = BF16


def _emit(tc, nc, xT_d, w1_d, wv_d, wp_d, out_d, dbg_d=None):
    ctx = ExitStack()
    with ctx:
        pers = ctx.enter_context(tc.tile_pool(name="pers", bufs=1))
        nc.gpsimd.load_library(library_config.attn)

        # ---------------- persistent SBUF ----------------
        xT_tiles = [pers.tile([128, T], IO_DT, name=f"xt{ci}") for ci in range(CI)]
        w1_sb = pers.tile([128, 4096], IO_DT, name="w1")    # wk0|wq0|wk1|wq1
        wv_sb = pers.tile([128, 2048], IO_DT, name="wv")
        wp_sb = pers.tile([128, 2048], IO_DT, name="wp")
        qT = [pers.tile([128, T], QKV_DT, name=f"qT{p}") for p in range(PAIRS)]
        kT = [pers.tile([128, T], QKV_DT, name=f"kT{p}") for p in range(PAIRS)]
        # v_aug per head: 16 t-blocks of 128 cols; cols 0..63 = v, 64..127 = 1.0
        v_sb = pers.tile([128, HPC * TB * 128], QKV_DT, name="v_sb")
        yT = [pers.tile([128, T], QKV_DT, name=f"yT{p}") for p in range(PAIRS)]
        mask_d = pers.tile([128, 128], P_DT, name="mask_d")

        # ---------------- DMAs (spread across sequencers) ----------------
        nc.sync.dma_start(w1_sb[:], w1_d[:])
        nc.scalar.dma_start(wv_sb[:], wv_d[:])
        for ci in range(CI):
            eng = (nc.sync, nc.scalar, nc.gpsimd)[ci % 3]
            eng.dma_start(xT_tiles[ci][:], xT_d[ts(ci, 128), :])
        nc.gpsimd.dma_start(wp_sb[:], wp_d[:])
        xT_sb = [t[:] for t in xT_tiles]

        # ones columns of v_aug (the AV matmul then emits the softmax
        # denominator broadcast across PSUM partitions 64..127 for free)
        nc.vector.memset(v_sb[:], 1.0)
        # 0/1 causal mask for the diagonal 128-block: (q - k >= 0)
        mask_f = pers.tile([128, 128], F32, name="mask_f")
        nc.gpsimd.memset(mask_f[:], 1.0)
        nc.gpsimd.affine_select(
            out=mask_f[:], in_=mask_f[:],
            compare_op=mybir.AluOpType.is_ge, fill=0.0,
            base=0, channel_multiplier=-1, pattern=[[1, 128]],
        )
        nc.vector.tensor_copy(mask_d[:], mask_f[:])

        def v_copy(psv, tb):
            # [128, 4h x 64d] PSUM -> per-head v_aug cols 0..63
            # (2D copies: multi-dim strided dst APs silently fail on DVE)
            for h in range(HPC):
                nc.vector.tensor_copy(
                    v_sb[:, h * 2048 + tb * 128: h * 2048 + tb * 128 + 64],
                    psv[:, ts(h, 64)],
                )

        # ---------------- ramp: ci-major qkv chains ----------------
        # k0/q0/k1/q1 chains for qc=0 plus v for tb 0..3, interleaved by ci
        # so each xT tile is consumed as soon as its DMA lands.
        kq_dst = [(kT[0], 0), (qT[0], 1), (kT[1], 2), (qT[1], 3)]
        with tc.tile_pool(name="psR", bufs=1, space=PSUM) as psR:
            pr_kq = [psR.tile([128, 512], F32, tag=f"rkq{j}", name=f"rkq{j}")
                     for j in range(4)]
            pr_v = [psR.tile([128, 256], F32, tag=f"rv{t}", name=f"rv{t}")
                    for t in range(4)]
            for ci in range(CI):
                for j in range(4):
                    nc.tensor.matmul(
                        pr_kq[j][:], w1_sb[:, j * 1024 + ci * 128:
                                           j * 1024 + ci * 128 + 128],
                        xT_sb[ci][:, 0:512],
                        start=(ci == 0), stop=(ci == CI - 1),
                    )
                for t in range(4):
                    nc.tensor.matmul(
                        pr_v[t][:], xT_sb[ci][:, ts(t, 128)],
                        wv_sb[:, ts(ci, 256)],
                        start=(ci == 0), stop=(ci == CI - 1),
                    )
            for (dst, j) in kq_dst:
                nc.vector.tensor_copy(dst[:, 0:512], pr_kq[j][:])
            for t in range(4):
                v_copy(pr_v[t], t)

        # ---------------- main pools ----------------
        with (
            tc.tile_pool(name="psS", bufs=2, space=PSUM) as psS,   # 4 banks
            tc.tile_pool(name="psY", bufs=1, space=PSUM) as psY,   # 2 banks
            tc.tile_pool(name="psF", bufs=2, space=PSUM) as psF,   # 2 banks
            tc.tile_pool(name="pP", bufs=4) as pP,
            tc.tile_pool(name="pN", bufs=3) as pN,
            tc.tile_pool(name="pO", bufs=2) as pO,
        ):
            # ---- filler units ----
            def qk_chain(p, qc, which):
                dst = kT[p] if which == "k" else qT[p]
                j = 2 * p + (0 if which == "k" else 1)
                ps = psF.tile([128, 512], F32, tag="f", name="fqk")
                for ci in range(CI):
                    nc.tensor.matmul(
                        ps[:], w1_sb[:, j * 1024 + ci * 128:
                                     j * 1024 + ci * 128 + 128],
                        xT_sb[ci][:, ts(qc, 512)],
                        start=(ci == 0), stop=(ci == CI - 1),
                    )
                nc.vector.tensor_copy(dst[:, ts(qc, 512)], ps[:])

            def v_chain(tb):
                psv = psF.tile([128, 256], F32, tag="f", name="fv")
                for ci in range(CI):
                    nc.tensor.matmul(
                        psv[:], xT_sb[ci][:, ts(tb, 128)],
                        wv_sb[:, ts(ci, 256)],
                        start=(ci == 0), stop=(ci == CI - 1),
                    )
                v_copy(psv, tb)

            ot_tiles = {}

            def proj_chunk(tb, cc):
                po = psF.tile([128, 512], F32, tag="f", name="fpo")
                for p in range(PAIRS):
                    nc.tensor.matmul(
                        po[:], yT[p][:, ts(tb, 128)],
                        wp_sb[:, p * 1024 + cc * 512: p * 1024 + cc * 512 + 512],
                        start=(p == 0), stop=(p == PAIRS - 1),
                    )
                if cc == 0:
                    ot_tiles[tb] = pO.tile([128, 1024], F32, tag="ot", name="ot")
                ot = ot_tiles[tb]
                nc.vector.tensor_copy(ot[:, ts(cc, 512)], po[:])
                if cc == 1:
                    eng = (nc.scalar, nc.gpsimd, nc.sync)[tb % 3]
                    eng.dma_start(out_d[ts(tb, 128), :], ot[:])
                    del ot_tiles[tb]

            filler_queue = []

            def emit_filler(n=1):
                for _ in range(n):
                    if not filler_queue:
                        return
                    kind, args = filler_queue.pop(0)
                    if kind == "qk":
                        qk_chain(*args)
                    elif kind == "v":
                        v_chain(*args)
                    else:
                        proj_chunk(*args)

            # ---- attention for (pair, qc): AV lags S by one step ----
            def attn(p, qc, budget=0):
                nkb = 4 * qc + 4
                ypt = [psY.tile([128, 512], F32, tag=f"y{hh}", name=f"y{hh}")
                       for hh in (0, 1)]
                pts = {}

                def s_step(kb):
                    col = max(0, (kb - 4 * qc) * 128)
                    sps = psS.tile([128, 1024], F32, tag="sps", name="sps")
                    for hh in (0, 1):
                        off = hh * 64
                        nc.tensor.matmul(
                            sps[:, hh * 512 + col: hh * 512 + 512],
                            kT[p][off:off + 64, ts(kb, 128)],
                            qT[p][off:off + 64,
                                  qc * 512 + col: (qc + 1) * 512],
                            start=True, stop=True,
                        )
                    pt = pP.tile([128, 1024], P_DT, tag="pt", name="pt")
                    if col == 0:
                        nc.scalar.activation(pt[:], sps[:], AF.Exp)
                    else:
                        for hh in (0, 1):
                            nc.scalar.activation(
                                pt[:, hh * 512 + col: hh * 512 + 512],
                                sps[:, hh * 512 + col: hh * 512 + 512],
                                AF.Exp)
                    if kb >= 4 * qc:   # mask the diagonal 128-block
                        for hh in (0, 1):
                            nc.vector.tensor_mul(
                                pt[:, hh * 512 + col: hh * 512 + col + 128],
                                pt[:, hh * 512 + col: hh * 512 + col + 128],
                                mask_d[:],
                            )
                    pts[kb] = (pt, col)

                def av_step(kb):
                    pt, col = pts.pop(kb)
                    for hh in (0, 1):
                        h = 2 * p + hh
                        nc.tensor.matmul(
                            ypt[hh][:, col:512],
                            v_sb[:, h * 2048 + kb * 128:
                                 h * 2048 + kb * 128 + 128],
                            pt[:, hh * 512 + col: hh * 512 + 512],
                            start=(kb == 0), stop=(kb == nkb - 1),
                        )

                s_step(0)
                for kb in range(nkb):
                    if kb + 1 < nkb:
                        s_step(kb + 1)
                    # spread `budget` filler units evenly across the steps
                    emit_filler(budget * (kb + 1) // nkb - budget * kb // nkb)
                    av_step(kb)
                # normalize: yT = ypt[0:64] * 1/l  (l = row 64; engines cannot
                # shift partition bases, so broadcast l via ACT copy + gpsimd)
                for hh in (0, 1):
                    l_sb = pN.tile([1, 512], F32, tag="l", name="l_sb")
                    nc.scalar.copy(l_sb[:], ypt[hh][64:65, :])
                    lb = pN.tile([64, 512], F32, tag="lb", name="lb")
                    nc.gpsimd.partition_broadcast(lb[:], l_sb[:])
                    rl = pN.tile([64, 512], F32, tag="rl", name="rl")
                    nc.vector.reciprocal_approx_fast(rl[:], lb[:])
                    nc.vector.tensor_mul(
                        yT[p][hh * 64: hh * 64 + 64, ts(qc, 512)],
                        ypt[hh][0:64, :], rl[:],
                    )

            # ---- phase 1: pair 0 attention; fillers = v + pair-1 qk ----
            # queue order must respect deps: v(tb) before AV step kb=tb of
            # attn(0, tb//4); k1/q1(qc) anytime before attn(1, qc).
            filler_queue += [("v", (4,)), ("v", (5,))]                  # qc0: 2
            filler_queue += [("v", (6,)), ("v", (7,)),
                             ("qk", (1, 1, "k")), ("qk", (1, 1, "q"))]  # qc1: 4
            filler_queue += [("v", (8,)), ("v", (9,)), ("v", (10,)),
                             ("v", (11,)), ("qk", (1, 2, "k"))]         # qc2: 5
            filler_queue += [("qk", (1, 2, "q")),
                             ("v", (12,)), ("v", (13,)), ("v", (14,)),
                             ("v", (15,)),
                             ("qk", (1, 3, "k")), ("qk", (1, 3, "q"))]  # qc3: 7
            p1_budget = [2, 4, 5, 7]

            for qc in range(NQC):
                if qc > 0:
                    qk_chain(0, qc, "k")
                    qk_chain(0, qc, "q")
                attn(0, qc, p1_budget[qc])

            # ---- phase 2: pair 1 attention; fillers = projection ----
            for qc in range(NQC):
                attn(1, qc, 0 if qc == 0 else 8)
                filler_queue += [("proj", (tb, cc))
                                 for tb in range(4 * qc, 4 * qc + 4)
                                 for cc in range(2)]
            emit_filler(len(filler_queue))

            if dbg_d is not None:
                nc.sync.dma_start(dbg_d[0][:], v_sb[:])
                nc.sync.dma_start(dbg_d[1][:, 0:T], yT[0][:])
                nc.sync.dma_start(dbg_d[1][:, T:2 * T], yT[1][:])
                nc.sync.dma_start(dbg_d[2][:, 0:T], qT[0][:])
                nc.sync.dma_start(dbg_d[2][:, T:2 * T], kT[0][:])


_NC_CACHE = None


def _build():
    global _NC_CACHE
    if _NC_CACHE is not None:
        return _NC_CACHE
    nc = bacc.Bacc("TRN2", target_bir_lowering=False, debug=False,
                   num_devices=N_CORES)
    xT_d = nc.dram_tensor("xT", [C, T], IO_DT, kind="ExternalInput")
    w1_d = nc.dram_tensor("w1", [128, 4096], IO_DT, kind="ExternalInput")
    wv_d = nc.dram_tensor("wv", [128, 2048], IO_DT, kind="ExternalInput")
    wp_d = nc.dram_tensor("wp", [128, 2048], IO_DT, kind="ExternalInput")
    out_d = nc.dram_tensor("out", [T, C], F32, kind="ExternalOutput")
    dbg_d = ([nc.dram_tensor("dbg", [128, HPC * TB * 128], IO_DT,
                             kind="ExternalOutput"),
              nc.dram_tensor("dbg2", [128, 2 * T], IO_DT,
                             kind="ExternalOutput"),
              nc.dram_tensor("dbg3", [128, 2 * T], IO_DT,
                             kind="ExternalOutput")] if DEBUG_V else None)

    with tile.TileContext(nc) as tc:
        _emit(tc, nc, xT_d, w1_d, wv_d, wp_d, out_d, dbg_d)
    nc.compile()
    _NC_CACHE = nc
    return nc


def _pack_pair(m):
    # [1024, 128] -> lhsT chunks layout [128, 8*128]
    return np.ascontiguousarray(
        m.reshape(CI, 128, 128).transpose(1, 0, 2).reshape(128, 1024))


def _io_np(a):
    import ml_dtypes
    return np.ascontiguousarray(a.astype(ml_dtypes.bfloat16))


def _in_maps(x, w_attn, w_proj):
    x = np.asarray(x, dtype=np.float32)
    w_attn = np.asarray(w_attn, dtype=np.float32)
    w_proj = np.asarray(w_proj, dtype=np.float32)
    xT = [_io_np(x[b].T) for b in range(B)]
    maps = []
    for core in range(N_CORES):
        b, g = core // HPC, core % HPC
        cols = slice(g * 256, (g + 1) * 256)
        wk_full = w_attn[:, 0 * C:1 * C][:, cols]
        wq_full = w_attn[:, 1 * C:2 * C][:, cols] * np.float32(1.0 / np.sqrt(HD))
        wv_full = w_attn[:, 2 * C:3 * C][:, cols]
        w1 = np.concatenate(
            [_pack_pair(m[:, p * 128:(p + 1) * 128])
             for p in range(PAIRS) for m in (wk_full, wq_full)], axis=1)
        wv_in = wv_full.reshape(CI, 128, 256).transpose(1, 0, 2).reshape(128, 2048)
        wp_in = (w_proj[g * 256:(g + 1) * 256, :]
                 .reshape(PAIRS, 128, 1024).transpose(1, 0, 2).reshape(128, 2048))
        maps.append({"xT": xT[b], "w1": _io_np(w1),
                     "wv": _io_np(wv_in), "wp": _io_np(wp_in)})
    return maps


def _assemble(results, b_proj):
    b_proj = np.asarray(b_proj, dtype=np.float32)
    out = np.zeros((B, T, C), dtype=np.float32)
    for core in range(N_CORES):
        out[core // HPC] += results[core]["out"]
    out += b_proj[None, None, :]
    return out


def kernel(x, w_attn, w_proj, b_proj):
    nc = _build()
    maps = _in_maps(x, w_attn, w_proj)
    res = run_bass_kernel_spmd(nc, maps, list(range(N_CORES)))
    return _assemble(res.results, b_proj)


def kernel_traced(x, w_attn, w_proj, b_proj):
    """Like kernel() but with NTFF tracing; returns (out, BassKernelResults)."""
    nc = _build()
    maps = _in_maps(x, w_attn, w_proj)
    res = run_bass_kernel_spmd(nc, maps, list(range(N_CORES)), trace=True)
    return _assemble(res.results, b_proj), res


# revision 27
# speedup vs baseline: 1.0257x; 1.0017x over previous
"""Trainium2 Bass kernel for causal self-attention (B=2, T=2048, C=1024, H=16).

Sharding: tensor-parallel over heads x data-parallel over batch.
Each of the 8 cores handles one (batch b, head-group g) pair: b = core // 4,
g = core % 4, where a head group is 4 consecutive heads (heads 4g..4g+3).

Per-core pipeline (v2 — software-pipelined, PE-saturating):
  Ramp: ci-major qkv chains (k0/q0/k1/q1 for qc0 + v tb0..3) so the PE
        computes while the 4MB xT streams in.
  Attention per (pair, qc), one k-block per step, AV lagging S by one step:
        PE order: S(kb+1) | filler | AV(kb); exp(kb+1) on ACT overlaps.
        Both heads' S^T live in one [128,1024] PSUM tile -> single exp.
  l-broadcast for free: v_aug columns 64..127 are 1.0, so AV's PSUM rows
        64..127 hold the softmax denominator replicated across partitions;
        normalize = DVE reciprocal + multiply straight out of PSUM.
  Fillers: remaining qk/v chains (phase 1) and projection chunks (phase 2)
        are interleaved between S and AV to hide exp latency and keep the
        PE p-state at max clock.
  Output projection partials summed on the host (the TP all-reduce), plus
        b_proj.
"""

import os
import numpy as np
from contextlib import ExitStack

DEBUG_V = os.environ.get("KBASS_DEBUG_V", "0") == "1"

import concourse.bass as bass
import concourse.tile as tile
from concourse import bacc, library_config, mybir
from concourse.bass import ts
from concourse.bass_utils import run_bass_kernel_spmd

F32 = mybir.dt.float32
BF16 = mybir.dt.bfloat16
AF = mybir.ActivationFunctionType
PSUM = bass.MemorySpace.PSUM

B, T, C, H = 2, 2048, 1024, 16
HD = C // H              # 64
HPC = 4                  # heads per core
PAIRS = 2                # head pairs per core
CI = C // 128            # 8 contraction chunks
TB = T // 128            # 16 t-blocks
NQC = T // 512           # 4 q-chunks
N_CORES = 8

IO_DT = BF16
QKV_DT = BF16
P_DT = BF16


def _emit(tc, nc, xT_d, w1_d, wv_d, wp_d, out_d, dbg_d=None):
    ctx = ExitStack()
    with ctx:
        pers = ctx.enter_context(tc.tile_pool(name="pers", bufs=1))
        nc.gpsimd.load_library(library_config.attn)

        # ---------------- persistent SBUF ----------------
        xT_tiles = [pers.tile([128, T], IO_DT, name=f"xt{ci}") for ci in range(CI)]
        # separate tiles per weight block -> fine-grained DMA deps
        w_sb = [pers.tile([128, 1024], IO_DT, name=f"w{j}") for j in range(4)]
        wv_sb = pers.tile([128, 2048], IO_DT, name="wv")
        wp_sb = pers.tile([128, 2048], IO_DT, name="wp")
        qT = [pers.tile([128, T], QKV_DT, name=f"qT{p}") for p in range(PAIRS)]
        kT = [pers.tile([128, T], QKV_DT, name=f"kT{p}") for p in range(PAIRS)]
        # v_aug per head: 16 t-blocks of 128 cols; cols 0..63 = v, 64..127 = 1.0
        v_sb = pers.tile([128, HPC * TB * 128], QKV_DT, name="v_sb")
        yT = [pers.tile([128, T], QKV_DT, name=f"yT{p}") for p in range(PAIRS)]
        mask_d = pers.tile([128, 128], P_DT, name="mask_d")

        # ---------------- DMAs (spread across sequencers) ----------------
        # ordered so the first ramp chain (wk0 + xT0) unblocks earliest
        dmas = [(w_sb[0], w1_d[:, 0:1024]), (xT_tiles[0], xT_d[ts(0, 128), :]),
                (w_sb[1], w1_d[:, 1024:2048]), (xT_tiles[1], xT_d[ts(1, 128), :]),
                (wv_sb, wv_d[:]), (xT_tiles[2], xT_d[ts(2, 128), :]),
                (w_sb[2], w1_d[:, 2048:3072]), (xT_tiles[3], xT_d[ts(3, 128), :]),
                (w_sb[3], w1_d[:, 3072:4096]), (xT_tiles[4], xT_d[ts(4, 128), :]),
                (xT_tiles[5], xT_d[ts(5, 128), :]),
                (xT_tiles[6], xT_d[ts(6, 128), :]),
                (xT_tiles[7], xT_d[ts(7, 128), :]), (wp_sb, wp_d[:])]
        for i, (dst, src) in enumerate(dmas):
            (nc.sync, nc.scalar, nc.gpsimd)[i % 3].dma_start(dst[:], src)
        xT_sb = [t[:] for t in xT_tiles]

        # ones columns of v_aug (the AV matmul then emits the softmax
        # denominator broadcast across PSUM partitions 64..127 for free)
        nc.vector.memset(v_sb[:], 1.0)
        # 0/1 causal mask for the diagonal 128-block: (q - k >= 0)
        mask_f = pers.tile([128, 128], F32, name="mask_f")
        nc.gpsimd.memset(mask_f[:], 1.0)
        nc.gpsimd.affine_select(
            out=mask_f[:], in_=mask_f[:],
            compare_op=mybir.AluOpType.is_ge, fill=0.0,
            base=0, channel_multiplier=-1, pattern=[[1, 128]],
        )
        nc.vector.tensor_copy(mask_d[:], mask_f[:])

        def v_copy(psv, tb):
            # [128, 4h x 64d] PSUM -> per-head v_aug cols 0..63
            # (2D copies: multi-dim strided dst APs silently fail on DVE)
            for h in range(HPC):
                nc.vector.tensor_copy(
                    v_sb[:, h * 2048 + tb * 128: h * 2048 + tb * 128 + 64],
                    psv[:, ts(h, 64)],
                )

        # ---------------- ramp: ci-major qkv chains ----------------
        # pair-0 k/q chains for qc 0 AND 1, plus v for tb 0..3, interleaved
        # by ci so each xT tile is consumed as soon as its DMA lands.
        ramp_kq = [(kT[0], 0, 0), (qT[0], 1, 0), (kT[0], 0, 1), (qT[0], 1, 1)]
        with tc.tile_pool(name="psR", bufs=1, space=PSUM) as psR:
            pr_kq = [psR.tile([128, 512], F32, tag=f"rkq{i}", name=f"rkq{i}")
                     for i in range(4)]
            pr_v = [psR.tile([128, 256], F32, tag=f"rv{t}", name=f"rv{t}")
                    for t in range(4)]
            for ci in range(CI):
                for i, (dst, j, qc) in enumerate(ramp_kq):
                    nc.tensor.matmul(
                        pr_kq[i][:], w_sb[j][:, ts(ci, 128)],
                        xT_sb[ci][:, ts(qc, 512)],
                        start=(ci == 0), stop=(ci == CI - 1),
                    )
                for t in range(4):
                    nc.tensor.matmul(
                        pr_v[t][:], xT_sb[ci][:, ts(t, 128)],
                        wv_sb[:, ts(ci, 256)],
                        start=(ci == 0), stop=(ci == CI - 1),
                    )
            for i, (dst, j, qc) in enumerate(ramp_kq):
                nc.vector.tensor_copy(dst[:, ts(qc, 512)], pr_kq[i][:])
            for t in range(4):
                v_copy(pr_v[t], t)

        # ---------------- main pools ----------------
        with (
            tc.tile_pool(name="psS", bufs=2, space=PSUM) as psS,   # 4 banks
            tc.tile_pool(name="psY", bufs=1, space=PSUM) as psY,   # 2 banks
            tc.tile_pool(name="psF", bufs=2, space=PSUM) as psF,   # 2 banks
            tc.tile_pool(name="pP", bufs=6) as pP,
            tc.tile_pool(name="pN", bufs=3) as pN,
            tc.tile_pool(name="pO", bufs=2) as pO,
        ):
            # ---- filler units ----
            def qk_chain(p, qc, which):
                dst = kT[p] if which == "k" else qT[p]
                j = 2 * p + (0 if which == "k" else 1)
                ps = psF.tile([128, 512], F32, tag="f", name="fqk")
                for ci in range(CI):
                    nc.tensor.matmul(
                        ps[:], w_sb[j][:, ts(ci, 128)],
                        xT_sb[ci][:, ts(qc, 512)],
                        start=(ci == 0), stop=(ci == CI - 1),
                    )
                nc.vector.tensor_copy(dst[:, ts(qc, 512)], ps[:])

            def v_chain(tb):
                psv = psF.tile([128, 256], F32, tag="f", name="fv")
                for ci in range(CI):
                    nc.tensor.matmul(
                        psv[:], xT_sb[ci][:, ts(tb, 128)],
                        wv_sb[:, ts(ci, 256)],
                        start=(ci == 0), stop=(ci == CI - 1),
                    )
                v_copy(psv, tb)

            ot_tiles = {}

            def proj_chunk(tb, cc):
                po = psF.tile([128, 512], F32, tag="f", name="fpo")
                for p in range(PAIRS):
                    nc.tensor.matmul(
                        po[:], yT[p][:, ts(tb, 128)],
                        wp_sb[:, p * 1024 + cc * 512: p * 1024 + cc * 512 + 512],
                        start=(p == 0), stop=(p == PAIRS - 1),
                    )
                if cc == 0:
                    ot_tiles[tb] = pO.tile([128, 1024], F32, tag="ot", name="ot")
                ot = ot_tiles[tb]
                nc.vector.tensor_copy(ot[:, ts(cc, 512)], po[:])
                if cc == 1:
                    eng = (nc.scalar, nc.gpsimd, nc.sync)[tb % 3]
                    eng.dma_start(out_d[ts(tb, 128), :], ot[:])
                    del ot_tiles[tb]

            filler_queue = []

            def emit_filler(n=1):
                for _ in range(n):
                    if not filler_queue:
                        return
                    kind, args = filler_queue.pop(0)
                    if kind == "qk":
                        qk_chain(*args)
                    elif kind == "v":
                        v_chain(*args)
                    else:
                        proj_chunk(*args)

            # ---- attention for (pair, qc): AV lags S by one step ----
            def attn(p, qc, budget=0):
                nkb = 4 * qc + 4
                ypt = [psY.tile([128, 512], F32, tag=f"y{hh}", name=f"y{hh}")
                       for hh in (0, 1)]
                pts = {}

                def s_step(kb):
                    col = max(0, (kb - 4 * qc) * 128)
                    sps = psS.tile([128, 1024], F32, tag="sps", name="sps")
                    for hh in (0, 1):
                        off = hh * 64
                        nc.tensor.matmul(
                            sps[:, hh * 512 + col: hh * 512 + 512],
                            kT[p][off:off + 64, ts(kb, 128)],
                            qT[p][off:off + 64,
                                  qc * 512 + col: (qc + 1) * 512],
                            start=True, stop=True,
                        )
                    pt = pP.tile([128, 1024], P_DT, tag="pt", name="pt")
                    if col == 0:
                        nc.scalar.activation(pt[:], sps[:], AF.Exp)
                    else:
                        for hh in (0, 1):
                            nc.scalar.activation(
                                pt[:, hh * 512 + col: hh * 512 + 512],
                                sps[:, hh * 512 + col: hh * 512 + 512],
                                AF.Exp)
                    if kb >= 4 * qc:   # mask the diagonal 128-block
                        for hh in (0, 1):
                            nc.vector.tensor_mul(
                                pt[:, hh * 512 + col: hh * 512 + col + 128],
                                pt[:, hh * 512 + col: hh * 512 + col + 128],
                                mask_d[:],
                            )
                    pts[kb] = (pt, col)

                def av_step(kb):
                    pt, col = pts.pop(kb)
                    for hh in (0, 1):
                        h = 2 * p + hh
                        nc.tensor.matmul(
                            ypt[hh][:, col:512],
                            v_sb[:, h * 2048 + kb * 128:
                                 h * 2048 + kb * 128 + 128],
                            pt[:, hh * 512 + col: hh * 512 + 512],
                            start=(kb == 0), stop=(kb == nkb - 1),
                        )

                s_step(0)
                for kb in range(nkb):
                    if kb + 1 < nkb:
                        s_step(kb + 1)
                    # spread `budget` filler units across steps, front-loaded
                    emit_filler(budget * (kb + 2) // (nkb + 1)
                                - budget * (kb + 1) // (nkb + 1))
                    av_step(kb)
                # lazy normalize: stage numerator + l out of PSUM quickly so
                # the ypt banks free for the next qc; then broadcast l (row 64;
                # engines cannot shift partition bases, so ACT copy + gpsimd),
                # reciprocal, and multiply off the critical path.
                stage = []
                for hh in (0, 1):
                    l_sb = pN.tile([1, 512], F32, tag="l", name="l_sb")
                    nc.scalar.copy(l_sb[:], ypt[hh][64:65, :])
                    num = pN.tile([64, 512], F32, tag=f"num{hh}", name="num")
                    nc.vector.tensor_copy(num[:], ypt[hh][0:64, :])
                    stage.append((l_sb, num))
                for hh in (0, 1):
                    l_sb, num = stage[hh]
                    lb = pN.tile([64, 512], F32, tag="lb", name="lb")
                    nc.gpsimd.partition_broadcast(lb[:], l_sb[:])
                    rl = pN.tile([64, 512], F32, tag="rl", name="rl")
                    nc.vector.reciprocal_approx_fast(rl[:], lb[:])
                    nc.vector.tensor_mul(
                        yT[p][hh * 64: hh * 64 + 64, ts(qc, 512)],
                        num[:], rl[:],
                    )

            # ---- phase 1: pair 0 attention; fillers = v + pair-1 qk ----
            # queue order must respect deps: v(tb) before AV step kb=tb of
            # attn(0, tb//4); k1/q1(qc) anytime before attn(1, qc).
            filler_queue += [("qk", (1, 0, "k")), ("qk", (1, 0, "q")),
                             ("v", (4,))]                               # qc0: 3
            filler_queue += [("v", (5,)), ("v", (6,)), ("v", (7,)),
                             ("v", (8,)), ("v", (9,))]                  # qc1: 5
            filler_queue += [("v", (10,)), ("v", (11,)),
                             ("qk", (1, 1, "k")), ("qk", (1, 1, "q")),
                             ("v", (12,)), ("v", (13,))]                # qc2: 6
            filler_queue += [("v", (14,)), ("v", (15,)),
                             ("qk", (1, 2, "k")), ("qk", (1, 2, "q"))]  # qc3: 4
            p1_budget = [3, 5, 6, 4]

            for qc in range(NQC):
                if qc > 1:   # qc 0/1 chains were produced by the ramp
                    qk_chain(0, qc, "k")
                    qk_chain(0, qc, "q")
                attn(0, qc, p1_budget[qc])
            # phase-transition cover for attn(1,0)'s PSUM-bank reuse
            qk_chain(1, 3, "k")
            qk_chain(1, 3, "q")

            # ---- phase 2: pair 1 attention; fillers = projection ----
            for qc in range(NQC):
                attn(1, qc, 8)
                filler_queue += [("proj", (tb, cc))
                                 for tb in range(4 * qc, 4 * qc + 4)
                                 for cc in range(2)]
            emit_filler(len(filler_queue))

            if dbg_d is not None:
                nc.sync.dma_start(dbg_d[0][:], v_sb[:])
                nc.sync.dma_start(dbg_d[1][:, 0:T], yT[0][:])
                nc.sync.dma_start(dbg_d[1][:, T:2 * T], yT[1][:])
                nc.sync.dma_start(dbg_d[2][:, 0:T], qT[0][:])
                nc.sync.dma_start(dbg_d[2][:, T:2 * T], kT[0][:])


_NC_CACHE = None


def _build():
    global _NC_CACHE
    if _NC_CACHE is not None:
        return _NC_CACHE
    nc = bacc.Bacc("TRN2", target_bir_lowering=False, debug=False,
                   num_devices=N_CORES)
    xT_d = nc.dram_tensor("xT", [C, T], IO_DT, kind="ExternalInput")
    w1_d = nc.dram_tensor("w1", [128, 4096], IO_DT, kind="ExternalInput")
    wv_d = nc.dram_tensor("wv", [128, 2048], IO_DT, kind="ExternalInput")
    wp_d = nc.dram_tensor("wp", [128, 2048], IO_DT, kind="ExternalInput")
    out_d = nc.dram_tensor("out", [T, C], F32, kind="ExternalOutput")
    dbg_d = ([nc.dram_tensor("dbg", [128, HPC * TB * 128], IO_DT,
                             kind="ExternalOutput"),
              nc.dram_tensor("dbg2", [128, 2 * T], IO_DT,
                             kind="ExternalOutput"),
              nc.dram_tensor("dbg3", [128, 2 * T], IO_DT,
                             kind="ExternalOutput")] if DEBUG_V else None)

    with tile.TileContext(nc) as tc:
        _emit(tc, nc, xT_d, w1_d, wv_d, wp_d, out_d, dbg_d)
    nc.compile()
    _NC_CACHE = nc
    return nc


def _pack_pair(m):
    # [1024, 128] -> lhsT chunks layout [128, 8*128]
    return np.ascontiguousarray(
        m.reshape(CI, 128, 128).transpose(1, 0, 2).reshape(128, 1024))


def _io_np(a):
    import ml_dtypes
    return np.ascontiguousarray(a.astype(ml_dtypes.bfloat16))


def _in_maps(x, w_attn, w_proj):
    x = np.asarray(x, dtype=np.float32)
    w_attn = np.asarray(w_attn, dtype=np.float32)
    w_proj = np.asarray(w_proj, dtype=np.float32)
    xT = [_io_np(x[b].T) for b in range(B)]
    maps = []
    for core in range(N_CORES):
        b, g = core // HPC, core % HPC
        cols = slice(g * 256, (g + 1) * 256)
        wk_full = w_attn[:, 0 * C:1 * C][:, cols]
        wq_full = w_attn[:, 1 * C:2 * C][:, cols] * np.float32(1.0 / np.sqrt(HD))
        wv_full = w_attn[:, 2 * C:3 * C][:, cols]
        w1 = np.concatenate(
            [_pack_pair(m[:, p * 128:(p + 1) * 128])
             for p in range(PAIRS) for m in (wk_full, wq_full)], axis=1)
        wv_in = wv_full.reshape(CI, 128, 256).transpose(1, 0, 2).reshape(128, 2048)
        wp_in = (w_proj[g * 256:(g + 1) * 256, :]
                 .reshape(PAIRS, 128, 1024).transpose(1, 0, 2).reshape(128, 2048))
        maps.append({"xT": xT[b], "w1": _io_np(w1),
                     "wv": _io_np(wv_in), "wp": _io_np(wp_in)})
    return maps


def _assemble(results, b_proj):
    b_proj = np.asarray(b_proj, dtype=np.float32)
    out = np.zeros((B, T, C), dtype=np.float32)
    for core in range(N_CORES):
        out[core // HPC] += results[core]["out"]
    out += b_proj[None, None, :]
    return out


def kernel(x, w_attn, w_proj, b_proj):
    nc = _build()
    maps = _in_maps(x, w_attn, w_proj)
    res = run_bass_kernel_spmd(nc, maps, list(range(N_CORES)))
    return _assemble(res.results, b_proj)


def kernel_traced(x, w_attn, w_proj, b_proj):
    """Like kernel() but with NTFF tracing; returns (out, BassKernelResults)."""
    nc = _build()
    maps = _in_maps(x, w_attn, w_proj)
    res = run_bass_kernel_spmd(nc, maps, list(range(N_CORES)), trace=True)
    return _assemble(res.results, b_proj), res


# revision 29
# speedup vs baseline: 1.0840x; 1.0568x over previous
"""Trainium2 Bass kernel for causal self-attention (B=2, T=2048, C=1024, H=16).

Sharding: tensor-parallel over heads x data-parallel over batch.
Each of the 8 cores handles one (batch b, head-group g) pair: b = core // 4,
g = core % 4, where a head group is 4 consecutive heads (heads 4g..4g+3).

Per-core pipeline (v2 — software-pipelined, PE-saturating):
  Ramp: ci-major qkv chains (k0/q0/k1/q1 for qc0 + v tb0..3) so the PE
        computes while the 4MB xT streams in.
  Attention per (pair, qc), one k-block per step, AV lagging S by one step:
        PE order: S(kb+1) | filler | AV(kb); exp(kb+1) on ACT overlaps.
        Both heads' S^T live in one [128,1024] PSUM tile -> single exp.
  l-broadcast for free: v_aug columns 64..127 are 1.0, so AV's PSUM rows
        64..127 hold the softmax denominator replicated across partitions;
        normalize = DVE reciprocal + multiply straight out of PSUM.
  Fillers: remaining qk/v chains (phase 1) and projection chunks (phase 2)
        are interleaved between S and AV to hide exp latency and keep the
        PE p-state at max clock.
  Output projection partials summed on the host (the TP all-reduce), plus
        b_proj.
"""

import os
import numpy as np
from contextlib import ExitStack

DEBUG_V = os.environ.get("KBASS_DEBUG_V", "0") == "1"

import concourse.bass as bass
import concourse.tile as tile
from concourse import bacc, library_config, mybir
from concourse.bass import ts
from concourse.bass_utils import run_bass_kernel_spmd

F32 = mybir.dt.float32
BF16 = mybir.dt.bfloat16
AF = mybir.ActivationFunctionType
PSUM = bass.MemorySpace.PSUM

B, T, C, H = 2, 2048, 1024, 16
HD = C // H              # 64
HPC = 4                  # heads per core
PAIRS = 2                # head pairs per core
CI = C // 128            # 8 contraction chunks
TB = T // 128            # 16 t-blocks
NQC = T // 512           # 4 q-chunks
N_CORES = 8

IO_DT = BF16
QKV_DT = BF16
P_DT = BF16


def _emit(tc, nc, xT_d, w1_d, wv_d, wp_d, out_d, dbg_d=None):
    ctx = ExitStack()
    with ctx:
        pers = ctx.enter_context(tc.tile_pool(name="pers", bufs=1))
        nc.gpsimd.load_library(library_config.attn)

        # ---------------- persistent SBUF ----------------
        xT_tiles = [pers.tile([128, T], IO_DT, name=f"xt{ci}") for ci in range(CI)]
        # separate tiles per weight block -> fine-grained DMA deps
        w_sb = [pers.tile([128, 1024], IO_DT, name=f"w{j}") for j in range(4)]
        wv_sb = pers.tile([128, 2048], IO_DT, name="wv")
        wp_sb = pers.tile([128, 2048], IO_DT, name="wp")
        qT = [pers.tile([128, T], QKV_DT, name=f"qT{p}") for p in range(PAIRS)]
        kT = [pers.tile([128, T], QKV_DT, name=f"kT{p}") for p in range(PAIRS)]
        # v_aug per head: 16 t-blocks of 128 cols; cols 0..63 = v, 64..127 = 1.0
        v_sb = pers.tile([128, HPC * TB * 128], QKV_DT, name="v_sb")
        yT = [pers.tile([128, T], QKV_DT, name=f"yT{p}") for p in range(PAIRS)]
        mask_d = pers.tile([128, 128], P_DT, name="mask_d")

        # ---------------- DMAs (spread across sequencers) ----------------
        # ordered so the first ramp chain (wk0 + xT0) unblocks earliest
        dmas = [(w_sb[0], w1_d[:, 0:1024]), (xT_tiles[0], xT_d[ts(0, 128), :]),
                (w_sb[1], w1_d[:, 1024:2048]), (xT_tiles[1], xT_d[ts(1, 128), :]),
                (wv_sb, wv_d[:]), (xT_tiles[2], xT_d[ts(2, 128), :]),
                (w_sb[2], w1_d[:, 2048:3072]), (xT_tiles[3], xT_d[ts(3, 128), :]),
                (w_sb[3], w1_d[:, 3072:4096]), (xT_tiles[4], xT_d[ts(4, 128), :]),
                (xT_tiles[5], xT_d[ts(5, 128), :]),
                (xT_tiles[6], xT_d[ts(6, 128), :]),
                (xT_tiles[7], xT_d[ts(7, 128), :]), (wp_sb, wp_d[:])]
        # gpsimd (SWDGE) delivery is ~10us late — keep inputs on sync/scalar
        for i, (dst, src) in enumerate(dmas):
            (nc.sync, nc.scalar)[i % 2].dma_start(dst[:], src)
        xT_sb = [t[:] for t in xT_tiles]

        # ones columns of v_aug (the AV matmul then emits the softmax
        # denominator broadcast across PSUM partitions 64..127 for free)
        nc.vector.memset(v_sb[:], 1.0)
        # 0/1 causal mask for the diagonal 128-block: (q - k >= 0)
        mask_f = pers.tile([128, 128], F32, name="mask_f")
        nc.gpsimd.memset(mask_f[:], 1.0)
        nc.gpsimd.affine_select(
            out=mask_f[:], in_=mask_f[:],
            compare_op=mybir.AluOpType.is_ge, fill=0.0,
            base=0, channel_multiplier=-1, pattern=[[1, 128]],
        )
        nc.vector.tensor_copy(mask_d[:], mask_f[:])

        def v_copy(psv, tb):
            # [128, 4h x 64d] PSUM -> per-head v_aug cols 0..63
            # (2D copies: multi-dim strided dst APs silently fail on DVE)
            for h in range(HPC):
                nc.vector.tensor_copy(
                    v_sb[:, h * 2048 + tb * 128: h * 2048 + tb * 128 + 64],
                    psv[:, ts(h, 64)],
                )

        # ---------------- ramp: ci-major qkv chains ----------------
        # pair-0 k/q chains for qc 0 AND 1, plus v for tb 0..3, interleaved
        # by ci so each xT tile is consumed as soon as its DMA lands.
        ramp_kq = [(kT[0], 0, 0), (qT[0], 1, 0), (kT[0], 0, 1), (qT[0], 1, 1)]
        with tc.tile_pool(name="psR", bufs=1, space=PSUM) as psR:
            pr_kq = [psR.tile([128, 512], F32, tag=f"rkq{i}", name=f"rkq{i}")
                     for i in range(4)]
            pr_v = [psR.tile([128, 256], F32, tag=f"rv{t}", name=f"rv{t}")
                    for t in range(4)]
            for ci in range(CI):
                for i, (dst, j, qc) in enumerate(ramp_kq):
                    nc.tensor.matmul(
                        pr_kq[i][:], w_sb[j][:, ts(ci, 128)],
                        xT_sb[ci][:, ts(qc, 512)],
                        start=(ci == 0), stop=(ci == CI - 1),
                    )
                for t in range(4):
                    nc.tensor.matmul(
                        pr_v[t][:], xT_sb[ci][:, ts(t, 128)],
                        wv_sb[:, ts(ci, 256)],
                        start=(ci == 0), stop=(ci == CI - 1),
                    )
            # copy order: unblock attn(0,0)'s S (kq qc0) then AV (v tb0) first
            nc.vector.tensor_copy(kT[0][:, 0:512], pr_kq[0][:])
            nc.vector.tensor_copy(qT[0][:, 0:512], pr_kq[1][:])
            v_copy(pr_v[0], 0)
            nc.vector.tensor_copy(kT[0][:, 512:1024], pr_kq[2][:])
            nc.vector.tensor_copy(qT[0][:, 512:1024], pr_kq[3][:])
            for t in range(1, 4):
                v_copy(pr_v[t], t)

        # ---------------- main pools ----------------
        with (
            tc.tile_pool(name="psS", bufs=2, space=PSUM) as psS,   # 4 banks
            tc.tile_pool(name="psY", bufs=1, space=PSUM) as psY,   # 2 banks
            tc.tile_pool(name="psF", bufs=2, space=PSUM) as psF,   # 2 banks
            tc.tile_pool(name="pP", bufs=6) as pP,
            tc.tile_pool(name="pN", bufs=3) as pN,
            tc.tile_pool(name="pO", bufs=2) as pO,
        ):
            # ---- filler units ----
            def qk_chain(p, qc, which):
                dst = kT[p] if which == "k" else qT[p]
                j = 2 * p + (0 if which == "k" else 1)
                ps = psF.tile([128, 512], F32, tag="f", name="fqk")
                for ci in range(CI):
                    nc.tensor.matmul(
                        ps[:], w_sb[j][:, ts(ci, 128)],
                        xT_sb[ci][:, ts(qc, 512)],
                        start=(ci == 0), stop=(ci == CI - 1),
                    )
                nc.vector.tensor_copy(dst[:, ts(qc, 512)], ps[:])

            def v_chain(tb):
                psv = psF.tile([128, 256], F32, tag="f", name="fv")
                for ci in range(CI):
                    nc.tensor.matmul(
                        psv[:], xT_sb[ci][:, ts(tb, 128)],
                        wv_sb[:, ts(ci, 256)],
                        start=(ci == 0), stop=(ci == CI - 1),
                    )
                v_copy(psv, tb)

            ot_tiles = {}

            def proj_chunk(tb, cc):
                po = psF.tile([128, 512], F32, tag="f", name="fpo")
                for p in range(PAIRS):
                    nc.tensor.matmul(
                        po[:], yT[p][:, ts(tb, 128)],
                        wp_sb[:, p * 1024 + cc * 512: p * 1024 + cc * 512 + 512],
                        start=(p == 0), stop=(p == PAIRS - 1),
                    )
                if cc == 0:
                    ot_tiles[tb] = pO.tile([128, 1024], F32, tag="ot", name="ot")
                ot = ot_tiles[tb]
                nc.vector.tensor_copy(ot[:, ts(cc, 512)], po[:])
                if cc == 1:
                    eng = (nc.scalar, nc.gpsimd, nc.sync)[tb % 3]
                    eng.dma_start(out_d[ts(tb, 128), :], ot[:])
                    del ot_tiles[tb]

            filler_queue = []

            def emit_filler(n=1):
                for _ in range(n):
                    if not filler_queue:
                        return
                    kind, args = filler_queue.pop(0)
                    if kind == "qk":
                        qk_chain(*args)
                    elif kind == "v":
                        v_chain(*args)
                    else:
                        proj_chunk(*args)

            # ---- attention for (pair, qc): AV lags S by one step ----
            def attn(p, qc, budget=0):
                nkb = 4 * qc + 4
                ypt = [psY.tile([128, 512], F32, tag=f"y{hh}", name=f"y{hh}")
                       for hh in (0, 1)]
                pts = {}

                def s_step(kb):
                    col = max(0, (kb - 4 * qc) * 128)
                    sps = psS.tile([128, 1024], F32, tag="sps", name="sps")
                    for hh in (0, 1):
                        off = hh * 64
                        nc.tensor.matmul(
                            sps[:, hh * 512 + col: hh * 512 + 512],
                            kT[p][off:off + 64, ts(kb, 128)],
                            qT[p][off:off + 64,
                                  qc * 512 + col: (qc + 1) * 512],
                            start=True, stop=True,
                        )
                    pt = pP.tile([128, 1024], P_DT, tag="pt", name="pt")
                    if col == 0:
                        nc.scalar.activation(pt[:], sps[:], AF.Exp)
                    else:
                        for hh in (0, 1):
                            nc.scalar.activation(
                                pt[:, hh * 512 + col: hh * 512 + 512],
                                sps[:, hh * 512 + col: hh * 512 + 512],
                                AF.Exp)
                    if kb >= 4 * qc:   # mask the diagonal 128-block
                        for hh in (0, 1):
                            nc.vector.tensor_mul(
                                pt[:, hh * 512 + col: hh * 512 + col + 128],
                                pt[:, hh * 512 + col: hh * 512 + col + 128],
                                mask_d[:],
                            )
                    pts[kb] = (pt, col)

                def av_step(kb):
                    pt, col = pts.pop(kb)
                    for hh in (0, 1):
                        h = 2 * p + hh
                        nc.tensor.matmul(
                            ypt[hh][:, col:512],
                            v_sb[:, h * 2048 + kb * 128:
                                 h * 2048 + kb * 128 + 128],
                            pt[:, hh * 512 + col: hh * 512 + 512],
                            start=(kb == 0), stop=(kb == nkb - 1),
                        )

                s_step(0)
                for kb in range(nkb):
                    if kb + 1 < nkb:
                        s_step(kb + 1)
                    # spread `budget` filler units across steps, front-loaded
                    emit_filler(budget * (kb + 2) // (nkb + 1)
                                - budget * (kb + 1) // (nkb + 1))
                    av_step(kb)
                # lazy normalize: stage numerator + l out of PSUM quickly so
                # the ypt banks free for the next qc; then broadcast l (row 64;
                # engines cannot shift partition bases, so ACT copy + gpsimd),
                # reciprocal, and multiply off the critical path.
                stage = []
                for hh in (0, 1):
                    l_sb = pN.tile([1, 512], F32, tag="l", name="l_sb")
                    nc.scalar.copy(l_sb[:], ypt[hh][64:65, :])
                    num = pN.tile([64, 512], F32, tag=f"num{hh}", name="num")
                    nc.vector.tensor_copy(num[:], ypt[hh][0:64, :])
                    stage.append((l_sb, num))
                for hh in (0, 1):
                    l_sb, num = stage[hh]
                    lb = pN.tile([64, 512], F32, tag="lb", name="lb")
                    nc.gpsimd.partition_broadcast(lb[:], l_sb[:])
                    rl = pN.tile([64, 512], F32, tag="rl", name="rl")
                    nc.vector.reciprocal_approx_fast(rl[:], lb[:])
                    nc.vector.tensor_mul(
                        yT[p][hh * 64: hh * 64 + 64, ts(qc, 512)],
                        num[:], rl[:],
                    )

            # ---- phase 1: pair 0 attention; fillers = v + pair-1 qk ----
            # queue order must respect deps: v(tb) before AV step kb=tb of
            # attn(0, tb//4); k1/q1(qc) anytime before attn(1, qc).
            filler_queue += [("qk", (1, 0, "k")), ("qk", (1, 0, "q")),
                             ("v", (4,))]                               # qc0: 3
            filler_queue += [("v", (5,)), ("v", (6,)), ("v", (7,)),
                             ("v", (8,)), ("v", (9,))]                  # qc1: 5
            filler_queue += [("v", (10,)), ("v", (11,)),
                             ("qk", (1, 1, "k")), ("qk", (1, 1, "q")),
                             ("v", (12,)), ("v", (13,))]                # qc2: 6
            filler_queue += [("v", (14,)), ("v", (15,)),
                             ("qk", (1, 2, "k")), ("qk", (1, 2, "q"))]  # qc3: 4
            p1_budget = [3, 5, 6, 4]

            for qc in range(NQC):
                if qc > 1:   # qc 0/1 chains were produced by the ramp
                    qk_chain(0, qc, "k")
                    qk_chain(0, qc, "q")
                attn(0, qc, p1_budget[qc])
            # phase-transition cover for attn(1,0)'s PSUM-bank reuse
            qk_chain(1, 3, "k")
            qk_chain(1, 3, "q")

            # ---- phase 2: pair 1 attention; fillers = projection ----
            for qc in range(NQC):
                attn(1, qc, 8)
                filler_queue += [("proj", (tb, cc))
                                 for tb in range(4 * qc, 4 * qc + 4)
                                 for cc in range(2)]
            emit_filler(len(filler_queue))

            if dbg_d is not None:
                nc.sync.dma_start(dbg_d[0][:], v_sb[:])
                nc.sync.dma_start(dbg_d[1][:, 0:T], yT[0][:])
                nc.sync.dma_start(dbg_d[1][:, T:2 * T], yT[1][:])
                nc.sync.dma_start(dbg_d[2][:, 0:T], qT[0][:])
                nc.sync.dma_start(dbg_d[2][:, T:2 * T], kT[0][:])


_NC_CACHE = None


def _build():
    global _NC_CACHE
    if _NC_CACHE is not None:
        return _NC_CACHE
    nc = bacc.Bacc("TRN2", target_bir_lowering=False, debug=False,
                   num_devices=N_CORES)
    xT_d = nc.dram_tensor("xT", [C, T], IO_DT, kind="ExternalInput")
    w1_d = nc.dram_tensor("w1", [128, 4096], IO_DT, kind="ExternalInput")
    wv_d = nc.dram_tensor("wv", [128, 2048], IO_DT, kind="ExternalInput")
    wp_d = nc.dram_tensor("wp", [128, 2048], IO_DT, kind="ExternalInput")
    out_d = nc.dram_tensor("out", [T, C], F32, kind="ExternalOutput")
    dbg_d = ([nc.dram_tensor("dbg", [128, HPC * TB * 128], IO_DT,
                             kind="ExternalOutput"),
              nc.dram_tensor("dbg2", [128, 2 * T], IO_DT,
                             kind="ExternalOutput"),
              nc.dram_tensor("dbg3", [128, 2 * T], IO_DT,
                             kind="ExternalOutput")] if DEBUG_V else None)

    with tile.TileContext(nc) as tc:
        _emit(tc, nc, xT_d, w1_d, wv_d, wp_d, out_d, dbg_d)
    nc.compile()
    _NC_CACHE = nc
    return nc


def _pack_pair(m):
    # [1024, 128] -> lhsT chunks layout [128, 8*128]
    return np.ascontiguousarray(
        m.reshape(CI, 128, 128).transpose(1, 0, 2).reshape(128, 1024))


def _io_np(a):
    import ml_dtypes
    return np.ascontiguousarray(a.astype(ml_dtypes.bfloat16))


def _in_maps(x, w_attn, w_proj):
    x = np.asarray(x, dtype=np.float32)
    w_attn = np.asarray(w_attn, dtype=np.float32)
    w_proj = np.asarray(w_proj, dtype=np.float32)
    xT = [_io_np(x[b].T) for b in range(B)]
    maps = []
    for core in range(N_CORES):
        b, g = core // HPC, core % HPC
        cols = slice(g * 256, (g + 1) * 256)
        wk_full = w_attn[:, 0 * C:1 * C][:, cols]
        wq_full = w_attn[:, 1 * C:2 * C][:, cols] * np.float32(1.0 / np.sqrt(HD))
        wv_full = w_attn[:, 2 * C:3 * C][:, cols]
        w1 = np.concatenate(
            [_pack_pair(m[:, p * 128:(p + 1) * 128])
             for p in range(PAIRS) for m in (wk_full, wq_full)], axis=1)
        wv_in = wv_full.reshape(CI, 128, 256).transpose(1, 0, 2).reshape(128, 2048)
        wp_in = (w_proj[g * 256:(g + 1) * 256, :]
                 .reshape(PAIRS, 128, 1024).transpose(1, 0, 2).reshape(128, 2048))
        maps.append({"xT": xT[b], "w1": _io_np(w1),
                     "wv": _io_np(wv_in), "wp": _io_np(wp_in)})
    return maps


def _assemble(results, b_proj):
    b_proj = np.asarray(b_proj, dtype=np.float32)
    out = np.zeros((B, T, C), dtype=np.float32)
    for core in range(N_CORES):
        out[core // HPC] += results[core]["out"]
    out += b_proj[None, None, :]
    return out


def kernel(x, w_attn, w_proj, b_proj):
    nc = _build()
    maps = _in_maps(x, w_attn, w_proj)
    res = run_bass_kernel_spmd(nc, maps, list(range(N_CORES)))
    return _assemble(res.results, b_proj)


def kernel_traced(x, w_attn, w_proj, b_proj):
    """Like kernel() but with NTFF tracing; returns (out, BassKernelResults)."""
    nc = _build()
    maps = _in_maps(x, w_attn, w_proj)
    res = run_bass_kernel_spmd(nc, maps, list(range(N_CORES)), trace=True)
    return _assemble(res.results, b_proj), res
